# revision 9
# baseline (speedup 1.0000x reference)
"""Trainium2 Bass kernel for nn_MultiHeadAttention_62766652064333.

Reference computation (per batch b, all 8 "heads" identical):
    Ql = Q @ Wq + bq;  Kl = K @ Wk + bk;  Vl = V @ Wv + bv
    scores = Ql @ Kl.T / sqrt(dm) + mask * (-1e9)
    att = softmax(scores, axis=-1)
    head = att @ Vl
    Y = tile(head, h) @ Wl + bl     == head @ (sum of h row-blocks of Wl) + bl
    att_ws = broadcast att over h

Sharding: data-parallel over batch — one batch per NeuronCore (8 cores).

Device dataflow (per core, "transposed" layouts so the PE contraction dim
is always on SBUF partitions; no on-device transposes needed):
    host supplies QT/KT/VT = X[b].T  (d-major [512, 1024])
    QlT[dout, q] = sum_di Wq[di, dout] * QT[di, q]      (lhsT = Wq blocks)
    KlT likewise;  Vl[k, dout] = sum_di VT[di, k] * Wv[di, dout]
    scoresT[k, q] = sum_d KlT[d, k] * QlT[d, q]
    exT = Exp(scoresT / sqrt(dm) + maskbias[k])         (ACT, mask as bias)
    denomR[*, q] = ones128.T @ exT       (replicated rows, for att store)
    denomQ[q]    = exT.T @ ones_col      (q on partitions, for Y scaling)
    att = exT * recip(denomR)  -> DMA (transposed; host untransposes)
    headT[d, q] = sum_k Vl[k, d] * exT[k, q]            (unnormalized)
    Y[q, :] = (headT.T @ Wlsum)[q, :] * recip(denomQ)[q] + bl

All big matmuls run as float32r (1 cycle/row on the PE for N>=256).
"""

import numpy as np
from contextlib import ExitStack

import concourse.bass as bass
import concourse.mybir as mybir
import concourse.tile as tile
from concourse import bacc
from concourse.bass_utils import run_bass_kernel_spmd

P = 128
DM = 512
H = 8
B = 8
SQ = 1024
SK = 1024
ND = DM // P     # 4 d-tiles of 128
NK = SK // P     # 8 k-tiles
NQ = SQ // P     # 8 q-tiles
NF = 512         # matmul moving free dim (fp32 max)
NH = SQ // NF    # 2 q-halves
F32 = mybir.dt.float32
F32R = mybir.dt.float32r
SM_SCALE = float(1.0 / np.sqrt(np.float32(DM)))


def build_bass():
    nc = bacc.Bacc("TRN2", target_bir_lowering=False, debug=False)
    AF = mybir.ActivationFunctionType

    qt_d = nc.dram_tensor("qt", [DM, SQ], F32R, kind="ExternalInput").ap()
    kt_d = nc.dram_tensor("kt", [DM, SK], F32R, kind="ExternalInput").ap()
    vt_d = nc.dram_tensor("vt", [DM, SK], F32R, kind="ExternalInput").ap()
    wq_d = nc.dram_tensor("wq", [DM, DM], F32R, kind="ExternalInput").ap()
    wk_d = nc.dram_tensor("wk", [DM, DM], F32R, kind="ExternalInput").ap()
    wv_d = nc.dram_tensor("wv", [DM, DM], F32R, kind="ExternalInput").ap()
    wl_d = nc.dram_tensor("wls", [DM, DM], F32R, kind="ExternalInput").ap()
    bq_d = nc.dram_tensor("bq", [P, ND], F32, kind="ExternalInput").ap()
    bk_d = nc.dram_tensor("bk", [P, ND], F32, kind="ExternalInput").ap()
    bv_d = nc.dram_tensor("bvr", [P, DM], F32, kind="ExternalInput").ap()
    bl_d = nc.dram_tensor("blr", [P, DM], F32, kind="ExternalInput").ap()
    mb_d = nc.dram_tensor("mb", [P, NK], F32, kind="ExternalInput").ap()
    ones_d = nc.dram_tensor("ones", [P, P], F32R, kind="ExternalInput").ap()

    att_d = nc.dram_tensor("attT", [SK, SQ], F32, kind="ExternalOutput").ap()
    y_d = nc.dram_tensor("y", [SQ, DM], F32, kind="ExternalOutput").ap()

    with tile.TileContext(nc) as tc, ExitStack() as ctx:
        consts = ctx.enter_context(tc.tile_pool(name="consts", bufs=1))
        wpool = ctx.enter_context(tc.tile_pool(name="wpool", bufs=1))
        bigp = ctx.enter_context(tc.tile_pool(name="bigp", bufs=6))
        exp_p = ctx.enter_context(tc.tile_pool(name="exp_p", bufs=1))
        stage = ctx.enter_context(tc.tile_pool(name="stage", bufs=3))
        pwork = ctx.enter_context(tc.tile_pool(name="pwork", bufs=5, space="PSUM"))
        pden = ctx.enter_context(tc.tile_pool(name="pden", bufs=2, space="PSUM"))
        pdnq = ctx.enter_context(tc.tile_pool(name="pdnq", bufs=1, space="PSUM"))

        # --- constants / small inputs ---
        ones128 = consts.tile([P, P], F32R, name="ones128", tag="ones128")
        nc.sync.dma_start(ones128[:], ones_d[:])
        bq_sb = consts.tile([P, ND], F32, name="bq_sb", tag="bq_sb")
        nc.sync.dma_start(bq_sb[:], bq_d[:])
        bk_sb = consts.tile([P, ND], F32, name="bk_sb", tag="bk_sb")
        nc.sync.dma_start(bk_sb[:], bk_d[:])
        bv_sb = consts.tile([P, DM], F32, name="bv_sb", tag="bv_sb")
        nc.sync.dma_start(bv_sb[:], bv_d[:])
        bl_sb = consts.tile([P, DM], F32, name="bl_sb", tag="bl_sb")
        nc.sync.dma_start(bl_sb[:], bl_d[:])
        mb_sb = consts.tile([P, NK], F32, name="mb_sb", tag="mb_sb")
        nc.sync.dma_start(mb_sb[:], mb_d[:])

        # --- weights ([128, 4, 512]: d_in on partitions) ---
        wq_sb = wpool.tile([P, ND, DM], F32R, name="wq_sb", tag="wq_sb")
        nc.sync.dma_start(wq_sb[:], wq_d.rearrange("(o p) f -> p o f", p=P))
        wk_sb = wpool.tile([P, ND, DM], F32R, name="wk_sb", tag="wk_sb")
        nc.sync.dma_start(wk_sb[:], wk_d.rearrange("(o p) f -> p o f", p=P))
        wv_sb = wpool.tile([P, ND, DM], F32R, name="wv_sb", tag="wv_sb")
        nc.sync.dma_start(wv_sb[:], wv_d.rearrange("(o p) f -> p o f", p=P))
        wl_sb = wpool.tile([P, ND, DM], F32R, name="wl_sb", tag="wl_sb")
        nc.sync.dma_start(wl_sb[:], wl_d.rearrange("(o p) f -> p o f", p=P))

        # --- big 16KB-class tiles share one rotating tag (6 slots) ---
        def big16(name):
            return bigp.tile([P, ND, SQ], F32R, name=name, tag="big16")

        qt_sb = big16("qt_sb")
        nc.sync.dma_start(qt_sb[:], qt_d.rearrange("(o p) q -> p o q", p=P))
        kt_sb = big16("kt_sb")
        nc.sync.dma_start(kt_sb[:], kt_d.rearrange("(o p) q -> p o q", p=P))
        vt_sb = big16("vt_sb")
        nc.sync.dma_start(vt_sb[:], vt_d.rearrange("(o p) q -> p o q", p=P))

        qlT = big16("qlT")
        klT = big16("klT")
        # Vl natural layout [k, dout]: [128, 8, 512]
        vl = bigp.tile([P, NK, DM], F32R, name="vl", tag="big16")

        ex = exp_p.tile([P, NK, SQ], F32R, name="ex", tag="ex")
        rc = consts.tile([P, SQ], F32, name="rc", tag="rc")
        rcq = consts.tile([P, NQ], F32, name="rcq", tag="rcq")

        def ps_tile(name):
            return pwork.tile([P, NF], F32, name=name, tag="ps")

        # --- Phase A: QlT / KlT projections ---
        for w_sb, x_sb, out_sb, b_sb, pname in (
            (wq_sb, qt_sb, qlT, bq_sb, "psql"),
            (wk_sb, kt_sb, klT, bk_sb, "pskl"),
        ):
            for dt in range(ND):
                pss = [ps_tile(f"{pname}_{dt}_{qh}") for qh in range(NH)]
                for di in range(ND):
                    for qh in range(NH):
                        nc.tensor.matmul(
                            pss[qh][:],
                            w_sb[:, di, dt * P:(dt + 1) * P],
                            x_sb[:, di, qh * NF:(qh + 1) * NF],
                            start=(di == 0),
                            stop=(di == ND - 1),
                        )
                for qh in range(NH):
                    nc.scalar.activation(
                        out_sb[:, dt, qh * NF:(qh + 1) * NF],
                        pss[qh][:],
                        AF.Identity,
                        bias=b_sb[:, dt:dt + 1],
                        scale=1.0,
                    )

        # --- Phase A': Vl = VT.T @ Wv + bv (natural [k, dout]) ---
        for kt_i in range(NK):
            ps = ps_tile(f"psvl_{kt_i}")
            for di in range(ND):
                nc.tensor.matmul(
                    ps[:],
                    vt_sb[:, di, kt_i * P:(kt_i + 1) * P],
                    wv_sb[:, di, :],
                    start=(di == 0),
                    stop=(di == ND - 1),
                )
            nc.vector.tensor_add(out=vl[:, kt_i, :], in0=ps[:], in1=bv_sb[:])

        # --- Phase B: scoresT -> exp -> denominators ---
        pd = [
            pden.tile([P, NF], F32, name=f"pd_{qh}", tag="pden") for qh in range(NH)
        ]
        for kt_i in range(NK):
            pss = [ps_tile(f"pssc_{kt_i}_{qh}") for qh in range(NH)]
            for di in range(ND):
                for qh in range(NH):
                    nc.tensor.matmul(
                        pss[qh][:],
                        klT[:, di, kt_i * P:(kt_i + 1) * P],
                        qlT[:, di, qh * NF:(qh + 1) * NF],
                        start=(di == 0),
                        stop=(di == ND - 1),
                    )
            for qh in range(NH):
                qs = slice(qh * NF, (qh + 1) * NF)
                nc.scalar.activation(
                    ex[:, kt_i, qs],
                    pss[qh][:],
                    AF.Exp,
                    bias=mb_sb[:, kt_i:kt_i + 1],
                    scale=SM_SCALE,
                )
                nc.tensor.matmul(
                    pd[qh][:],
                    ones128[:],
                    ex[:, kt_i, qs],
                    start=(kt_i == 0),
                    stop=(kt_i == NK - 1),
                )

        # per-q-partition denominator (for Y scaling). fp32r matmul needs a
        # wider dst pattern, so each qi writes 8 identical columns.
        denq = pdnq.tile([P, NQ, 8], F32, name="denq", tag="denq")
        for qi in range(NQ):
            for kt_i in range(NK):
                nc.tensor.matmul(
                    denq[:, qi, :],
                    ex[:, kt_i, qi * P:(qi + 1) * P],
                    ones128[:, 0:8],
                    start=(kt_i == 0),
                    stop=(kt_i == NK - 1),
                )

        # --- reciprocals of denominators ---
        for qh in range(NH):
            nc.vector.reciprocal(rc[:, qh * NF:(qh + 1) * NF], pd[qh][:])
        nc.vector.reciprocal(rcq[:], denq[:, :, 0])

        # --- normalize att into staging, stream out ---
        for kt_i in range(NK):
            att_st = stage.tile([P, SQ], F32, name=f"att_st_{kt_i}", tag="att_st")
            for qh in range(NH):
                qs = slice(qh * NF, (qh + 1) * NF)
                nc.vector.tensor_mul(
                    out=att_st[:, qs], in0=ex.bitcast(F32)[:, kt_i, qs], in1=rc[:, qs]
                )
            nc.sync.dma_start(att_d[kt_i * P:(kt_i + 1) * P, :], att_st[:])

        # --- Phase C: headT[d, q] = Vl.T @ exT (unnormalized) ---
        hT = big16("hT")
        for dt in range(ND):
            pss = [ps_tile(f"pshd_{dt}_{qh}") for qh in range(NH)]
            for kt_i in range(NK):
                for qh in range(NH):
                    nc.tensor.matmul(
                        pss[qh][:],
                        vl[:, kt_i, dt * P:(dt + 1) * P],
                        ex[:, kt_i, qh * NF:(qh + 1) * NF],
                        start=(kt_i == 0),
                        stop=(kt_i == NK - 1),
                    )
            for qh in range(NH):
                nc.scalar.activation(
                    hT[:, dt, qh * NF:(qh + 1) * NF],
                    pss[qh][:],
                    AF.Copy,
                )

        # --- Phase Y: Y[q, :] = (headT.T @ Wlsum) * rcq[q] + bl ---
        for qi in range(NQ):
            ps = ps_tile(f"psy_{qi}")
            for di in range(ND):
                nc.tensor.matmul(
                    ps[:],
                    hT[:, di, qi * P:(qi + 1) * P],
                    wl_sb[:, di, :],
                    start=(di == 0),
                    stop=(di == ND - 1),
                )
            y_sb = stage.tile([P, DM], F32, name=f"y_sb_{qi}", tag="y_sb")
            nc.vector.scalar_tensor_tensor(
                out=y_sb[:],
                in0=ps[:],
                scalar=rcq[:, qi:qi + 1],
                in1=bl_sb[:],
                op0=mybir.AluOpType.mult,
                op1=mybir.AluOpType.add,
            )
            nc.sync.dma_start(y_d[qi * P:(qi + 1) * P, :], y_sb[:])

    nc.compile()
    return nc


_NC_CACHE = {}


def get_nc():
    if "nc" not in _NC_CACHE:
        _NC_CACHE["nc"] = build_bass()
    return _NC_CACHE["nc"]


def prepare_in_maps(Q, K, V, mask, Wq, bq, Wk, bk, Wv, bv, Wl, bl):
    f = lambda a: np.ascontiguousarray(np.asarray(a, dtype=np.float32))
    Q, K, V = f(Q), f(K), f(V)
    Wq, Wk, Wv, Wl = f(Wq), f(Wk), f(Wv), f(Wl)
    bq, bk, bv, bl = f(bq), f(bk), f(bv), f(bl)
    mask = np.asarray(mask)

    wls = np.ascontiguousarray(
        Wl.reshape(H, DM, DM).sum(axis=0, dtype=np.float64).astype(np.float32)
    )
    bq2 = np.ascontiguousarray(bq.reshape(ND, P).T)       # [128, 4]
    bk2 = np.ascontiguousarray(bk.reshape(ND, P).T)
    bvr = np.ascontiguousarray(np.broadcast_to(bv, (P, DM)))  # replicated rows
    blr = np.ascontiguousarray(np.broadcast_to(bl, (P, DM)))

    in_maps = []
    for b in range(B):
        mb = (mask[b, 0].astype(np.float32) * np.float32(-1e9))
        in_maps.append(
            {
                "qt": np.ascontiguousarray(Q[b].T),
                "kt": np.ascontiguousarray(K[b].T),
                "vt": np.ascontiguousarray(V[b].T),
                "wq": Wq,
                "wk": Wk,
                "wv": Wv,
                "wls": wls,
                "bq": bq2,
                "bk": bk2,
                "bvr": bvr,
                "blr": blr,
                "mb": np.ascontiguousarray(mb.reshape(NK, P).T),  # [128, 8]
                "ones": np.ones((P, P), dtype=np.float32),
            }
        )
    return in_maps


def postprocess(results):
    Y = np.stack([np.asarray(results[b]["y"]) for b in range(B)])
    att = np.stack([np.asarray(results[b]["attT"]).T for b in range(B)])
    att_ws = np.broadcast_to(att[:, None], (B, H, SQ, SK))
    return Y, att_ws


def kernel(Q, K, V, mask, Wq, bq, Wk, bk, Wv, bv, Wl, bl):
    nc = get_nc()
    in_maps = prepare_in_maps(Q, K, V, mask, Wq, bq, Wk, bk, Wv, bv, Wl, bl)
    res = run_bass_kernel_spmd(nc, in_maps, list(range(B)))
    return postprocess(res.results)
